# revision 8
# baseline (speedup 1.0000x reference)
"""Trainium2 Bass kernel for nn_Attention_28905129902499.

Dense transformer attention block (q/k/v proj + RoPE + causal GQA attention
+ o_proj), B=1, S=2048, HIDDEN=2048, 32 q heads / 8 kv heads, head_dim 64.

Sharding: tensor-parallel over heads across 8 NeuronCores. Core c owns
q heads 4c..4c+3 and kv head c. Each core computes its partial
out_c = attn_c @ wo[:, c*256:(c+1)*256].T  (shape [S, H]); the host sums the
8 partials (the tensor-parallel all-reduce) and returns the full output.

Device-side layout notes (per core):
  - All matmuls run in bf16 with fp32 PSUM accumulation.
  - q/k are produced *transposed*: qT/kT [d, s] with head_dim on partitions,
    so attention scores are computed directly transposed, scoresT[k, s] =
    kT.T @ qT, with no on-chip transposes of the big S x S tensors.
  - softmax runs without max subtraction (scores are O(+-6) here, exp is
    safe in fp32) and the denominators come for free out of the PV matmul:
    V is extended with 64 all-ones columns so out rows carry sum(exp).
  - RoPE cos/sin are computed on device from position_ids: freqs via a
    K=1 fp32 outer-product matmul, Cody-Waite range reduction on DVE,
    sin/cos on the ACT spline engine.
"""

import sys
import types
from contextlib import ExitStack

import numpy as np
import ml_dtypes

for _p in ("/opt/trn_rl_repo", "/root/.axon_site/_ro/trn_rl_repo"):
    if _p not in sys.path:
        sys.path.append(_p)

import concourse.bass as bass
import concourse.tile as tile
import concourse.mybir as mybir
from concourse.bass_utils import run_bass_kernel_spmd

dt = mybir.dt
AF = mybir.ActivationFunctionType
ALU = mybir.AluOpType
bf16 = ml_dtypes.bfloat16

# ---------------------------------------------------------------- constants
S = 2048          # sequence length
H = 2048          # hidden size
NH = 32           # query heads
NKV = 8           # kv heads
D = 64            # head dim
G = NH // NKV     # 4 query heads per kv head
N_CORES = 8
DQ = G * D        # 256 local q dims per core
MQKV = DQ + 2 * D   # 384 fused qkv output dims per core
KT = H // 128     # 16 contraction tiles
NS = S // 512     # 4 sequence chunks of 512
KB = S // 128     # 16 key blocks of 128
SCALE = 1.0 / np.sqrt(D)
ROPE_BASE = 10000.0

TWO_PI = 2.0 * np.pi
# Cody-Waite split of 2*pi for fp32 range reduction
_C1 = float(np.float32(np.ldexp(np.round(np.ldexp(TWO_PI, 11)), -11)))
_C2 = float(np.float32(np.ldexp(np.round(np.ldexp(TWO_PI - _C1, 23)), -23)))


def _split_multi_waits(nc):
    """The walrus build in this container accepts only ONE sync-wait per
    instruction; Tile emits more. Move extras onto same-engine NOPs placed
    immediately before the instruction (same-engine streams are in-order, so
    this is semantically identical)."""
    for bb in nc.main_func.blocks:
        insts = bb.instructions
        i = 0
        while i < len(insts):
            ins = insts[i]
            si = ins.sync_info
            waits = list(si.on_wait) if si is not None else []
            if len(waits) > 1:
                for w in waits[:-1]:
                    nop = mybir.InstNoOp(
                        name=nc.get_next_instruction_name(),
                        engine=ins.engine,
                        bass_nofuse=True,
                        sync_info=mybir.SyncInfo(on_wait=[w], on_update=[]),
                    )
                    nc.register_instruction(nop, overwrite=True)
                    insts.insert(i, nop)
                    i += 1
                ins.sync_info = mybir.SyncInfo(
                    on_wait=[waits[-1]], on_update=list(si.on_update)
                )
            i += 1


def _install_profile_hook():
    """Register the NTFF profile hook the agent image's antenv lacks, so
    run_bass_kernel_spmd(trace=True) can return HW exec times."""
    try:
        import antenv.axon_hooks  # noqa: F401
        return
    except ImportError:
        pass
    hook = None
    try:
        from trn_agent_boot.trn_boot import _ntff_profile_via_ctypes
        hook = _ntff_profile_via_ctypes("/opt/axon/libaxon_pjrt.so")
    except Exception:
        hook = None
    m = types.ModuleType("antenv.axon_hooks")
    m.get_axon_ntff_profile_hook = lambda: hook
    m.set_axon_ntff_profile_hook = lambda h: None
    sys.modules["antenv.axon_hooks"] = m


# ---------------------------------------------------------------- program
def build_program():
    nc = bass.Bass()

    xT = nc.declare_dram_parameter("xT", [H, S], dt.float32, isOutput=False)
    wqkvT = nc.declare_dram_parameter("wqkvT", [H, MQKV], dt.float32, isOutput=False)
    woT = nc.declare_dram_parameter("woT", [DQ, H], dt.float32, isOutput=False)
    posr = nc.declare_dram_parameter("posr", [1, S], dt.float32, isOutput=False)
    invf = nc.declare_dram_parameter("invf", [1, 32], dt.float32, isOutput=False)
    rt2 = nc.declare_dram_parameter("rt2", [128, 128], dt.float32, isOutput=False)
    poutT = nc.declare_dram_parameter("poutT", [H, S], dt.bfloat16, isOutput=True)

    with tile.TileContext(nc) as tc, ExitStack() as stack:
        # ---------------- persistent pools ----------------
        const_pool = stack.enter_context(tc.tile_pool(name="const", bufs=1))
        trig_pool = stack.enter_context(tc.tile_pool(name="trig", bufs=1))

        # pi/2 per-partition bias vector for cos-via-sin
        pi2_bias = const_pool.tile([128, 1], dt.float32, tag="pi2")
        nc.gpsimd.memset(pi2_bias[:], float(np.pi / 2))

        # rope rotation matrix
        rt_f = const_pool.tile([128, 128], dt.float32, tag="rtf")
        nc.sync.dma_start(rt_f[:], rt2[:])
        rt_b = const_pool.tile([128, 128], dt.bfloat16, tag="rtb")
        nc.gpsimd.tensor_copy(rt_b[:], rt_f[:])

        # position/frequency rows
        pos_sb = const_pool.tile([1, S], dt.float32, tag="pos")
        nc.sync.dma_start(pos_sb[:], posr[:])
        invf_sb = const_pool.tile([1, 32], dt.float32, tag="invf")
        nc.sync.dma_start(invf_sb[:], invf[:])

        # ---------------- RoPE trig tables ----------------
        # freqs in chunk-stacked layout [ (chunk c, f) , 512 ]:
        #   partition 32c+f  = inv_freq[f] * pos[512c + j]
        cos_rep = trig_pool.tile([128, S], dt.float32, tag="cosr")
        sin_rep = trig_pool.tile([128, S], dt.float32, tag="sinr")

        with tc.tile_pool(name="trig_psum", bufs=1, space="PSUM") as tpsum, \
             tc.tile_pool(name="trig_sc", bufs=1) as tsc:
            fq = tpsum.tile([128, 512], dt.float32, tag="fq")
            for c in range(4):
                nc.tensor.matmul(
                    fq[32 * c:32 * (c + 1), :],
                    invf_sb[:],
                    pos_sb[:, 512 * c:512 * (c + 1)],
                    start=True, stop=True,
                    tile_position=(0, 32 * c),
                )
            f_sb = tsc.tile([128, 512], dt.float32, tag="fsb")
            nc.vector.tensor_copy(f_sb[:], fq[:])

            # sin: k = round(f / 2pi); r = f - k*c1 - k*c2; sin(r)
            y = tsc.tile([128, 512], dt.float32, tag="y")
            nc.vector.tensor_scalar(out=y[:], in0=f_sb[:], scalar1=1.0 / TWO_PI,
                                    scalar2=None, op0=ALU.mult)
            ki = tsc.tile([128, 512], dt.int32, tag="ki")
            nc.vector.tensor_copy(ki[:], y[:])
            kf = tsc.tile([128, 512], dt.float32, tag="kf")
            nc.vector.tensor_copy(kf[:], ki[:])
            t1 = tsc.tile([128, 512], dt.float32, tag="t1")
            nc.vector.tensor_scalar(out=t1[:], in0=kf[:], scalar1=_C1,
                                    scalar2=None, op0=ALU.mult)
            r1 = tsc.tile([128, 512], dt.float32, tag="r1")
            nc.vector.tensor_tensor(out=r1[:], in0=f_sb[:], in1=t1[:], op=ALU.subtract)
            nc.vector.tensor_scalar(out=t1[:], in0=kf[:], scalar1=_C2,
                                    scalar2=None, op0=ALU.mult)
            nc.vector.tensor_tensor(out=r1[:], in0=r1[:], in1=t1[:], op=ALU.subtract)
            sin_c = tsc.tile([128, 512], dt.float32, tag="sinc")
            nc.scalar.activation(sin_c[:], r1[:], AF.Sin)

            # cos(f) = sin(f + pi/2 - kc*2pi), kc = round(f/2pi + 1/4)
            nc.vector.tensor_scalar(out=y[:], in0=y[:], scalar1=0.25,
                                    scalar2=None, op0=ALU.add)
            nc.vector.tensor_copy(ki[:], y[:])
            nc.vector.tensor_copy(kf[:], ki[:])
            nc.vector.tensor_scalar(out=t1[:], in0=kf[:], scalar1=_C1,
                                    scalar2=None, op0=ALU.mult)
            nc.vector.tensor_tensor(out=r1[:], in0=f_sb[:], in1=t1[:], op=ALU.subtract)
            nc.vector.tensor_scalar(out=t1[:], in0=kf[:], scalar1=_C2,
                                    scalar2=None, op0=ALU.mult)
            nc.vector.tensor_tensor(out=r1[:], in0=r1[:], in1=t1[:], op=ALU.subtract)
            cos_c = tsc.tile([128, 512], dt.float32, tag="cosc")
            nc.scalar.activation(cos_c[:], r1[:], AF.Sin, bias=pi2_bias[:])

            # replicate [ (c, f), 512 ] -> [ f rep x4 , (c, 512) ]
            for c in range(4):
                for i in range(4):
                    nc.sync.dma_start(
                        cos_rep[32 * i:32 * (i + 1), 512 * c:512 * (c + 1)],
                        cos_c[32 * c:32 * (c + 1), :])
                    nc.sync.dma_start(
                        sin_rep[32 * i:32 * (i + 1), 512 * c:512 * (c + 1)],
                        sin_c[32 * c:32 * (c + 1), :])

        # ---------------- load x / weights, convert to bf16 ----------------
        proj_pool = stack.enter_context(tc.tile_pool(name="proj", bufs=1))
        wqkv_b = [proj_pool.tile([128, MQKV], dt.bfloat16, tag=f"wqkv{k}", name=f"wqkv{k}")
                  for k in range(KT)]
        wo_b = [proj_pool.tile([128, S], dt.bfloat16, tag=f"wo{k}", name=f"wo{k}") for k in range(2)]
        xt_pool = stack.enter_context(tc.tile_pool(name="xtb", bufs=1))
        xt_b = [xt_pool.tile([128, S], dt.bfloat16, tag=f"xt{k}", name=f"xtb{k}") for k in range(KT)]

        with tc.tile_pool(name="stage", bufs=3) as stage:
            for k in range(KT):
                wf = stage.tile([128, MQKV], dt.float32, tag="wstage")
                nc.sync.dma_start(wf[:], wqkvT[128 * k:128 * (k + 1), :])
                nc.gpsimd.tensor_copy(wqkv_b[k][:], wf[:])
            for k in range(2):
                wf2 = stage.tile([128, S], dt.float32, tag="wostage")
                nc.sync.dma_start(wf2[:], woT[128 * k:128 * (k + 1), :])
                nc.gpsimd.tensor_copy(wo_b[k][:], wf2[:])
            for k in range(KT):
                xf = stage.tile([128, S], dt.float32, tag="xstage")
                nc.sync.dma_start(xf[:], xT[128 * k:128 * (k + 1), :])
                nc.gpsimd.tensor_copy(xt_b[k][:], xf[:])

        # ---------------- fused QKV projection + RoPE ----------------
        # m=0: q heads 0,1 | m=1: q heads 2,3 | m=2: rows 0-63 kT, 64-127 vT
        att_pool = stack.enter_context(tc.tile_pool(name="att", bufs=1))
        qrope = [att_pool.tile([128, S], dt.bfloat16, tag=f"qrope{p}", name=f"qrope{p}")
                 for p in range(2)]
        krope = att_pool.tile([128, S], dt.bfloat16, tag="krope")
        # v_ext A: cols 0-63 v, 64-127 ones (pv rows 0-63, sums rows 64-127)
        # v_ext B: cols 0-63 ones, 64-127 v
        vextA = att_pool.tile([128, S], dt.bfloat16, tag="vextA")
        vextB = att_pool.tile([128, S], dt.bfloat16, tag="vextB")
        nc.gpsimd.memset(vextA[:], 1.0)
        nc.gpsimd.memset(vextB[:], 1.0)
        vT_sb = att_pool.tile([128, S], dt.bfloat16, tag="vTsb")

        with tc.tile_pool(name="qkv_psum", bufs=1, space="PSUM") as qpsum, \
             tc.tile_pool(name="rot_psum", bufs=2, space="PSUM") as rpsum, \
             tc.tile_pool(name="rope_sc", bufs=3) as rsc:
            for m in range(3):
                ps = qpsum.tile([128, S], dt.float32, tag="qkvps")
                for k in range(KT):
                    for n in range(NS):
                        nc.tensor.matmul(
                            ps[:, 512 * n:512 * (n + 1)],
                            wqkv_b[k][:, 128 * m:128 * (m + 1)],
                            xt_b[k][:, 512 * n:512 * (n + 1)],
                            start=(k == 0), stop=(k == KT - 1),
                        )
                nrows = 128 if m < 2 else 64
                for n in range(NS):
                    sl = slice(512 * n, 512 * (n + 1))
                    # qc = q * cos
                    qc = rsc.tile([128, 512], dt.float32, tag="qc")
                    nc.vector.tensor_tensor(out=qc[:nrows, :], in0=ps[:nrows, sl],
                                            in1=cos_rep[:nrows, sl], op=ALU.mult)
                    # raw copy for the rotation matmul
                    qraw = rsc.tile([128, 512], dt.bfloat16, tag="qraw")
                    nc.vector.tensor_copy(qraw[:nrows, :], ps[:nrows, sl])
                    rot = rpsum.tile([128, 512], dt.float32, tag="rot")
                    nc.tensor.matmul(rot[:nrows, :], rt_b[:nrows, :nrows],
                                     qraw[:nrows, :], start=True, stop=True)
                    # qs = rot(q) * sin ; qrope = qc + qs
                    qs = rsc.tile([128, 512], dt.float32, tag="qs")
                    nc.vector.tensor_tensor(out=qs[:nrows, :], in0=rot[:nrows, :],
                                            in1=sin_rep[:nrows, sl], op=ALU.mult)
                    dst = qrope[m] if m < 2 else krope
                    nc.vector.tensor_tensor(out=dst[:nrows, sl], in0=qc[:nrows, :],
                                            in1=qs[:nrows, :], op=ALU.add)
                    if m == 2:
                        # v rows: plain bf16 copy
                        nc.vector.tensor_copy(vT_sb[64:128, sl], ps[64:128, sl])

        # duplicate kT onto partitions 64-127 (for odd-head matmuls)
        nc.sync.dma_start(krope[64:128, :], krope[0:64, :])
        # transpose vT [64, S] into v_ext blocks [k(128), d(64)] per key block
        for kb in range(KB):
            nc.sync.dma_start_transpose(
                vextA[:, 128 * kb:128 * kb + 64],
                vT_sb[64:128, 128 * kb:128 * (kb + 1)])
        for kb in range(KB):
            nc.sync.dma_start(vextB[:, 128 * kb + 64:128 * (kb + 1)],
                              vextA[:, 128 * kb:128 * kb + 64])

        attnT = [att_pool.tile([128, S], dt.bfloat16, tag=f"attnT{p}", name=f"attnT{p}")
                 for p in range(2)]

        # ---------------- attention (per local q head) ----------------
        with tc.tile_pool(name="sc_psum", bufs=1, space="PSUM") as spsum, \
             tc.tile_pool(name="pv_psum", bufs=1, space="PSUM") as vpsum, \
             tc.tile_pool(name="exp_sb", bufs=2) as esb, \
             tc.tile_pool(name="norm_sb", bufs=4) as nsb:
            for h in range(4):
                pair = h // 2
                par = h % 2          # 0: even head (base 0), 1: odd (base 64)
                hlo, hhi = 64 * par, 64 * par + 64
                vext = vextA if par == 0 else vextB
                pvrow = slice(0, 64) if par == 0 else slice(64, 128)
                smrow = slice(64, 128) if par == 0 else slice(0, 64)
                pvs = [vpsum.tile([128, 512], dt.float32, tag=f"pv{q}", name=f"pv{q}")
                       for q in range(NS)]
                for kb in range(KB):
                    qlo = kb // 4        # first 512-chunk that attends to kb
                    qlen = 512 * (NS - qlo)
                    sc = spsum.tile([128, 2048], dt.float32, tag="scps")
                    for q in range(qlo, NS):
                        nc.tensor.matmul(
                            sc[:, 512 * (q - qlo):512 * (q - qlo + 1)],
                            krope[hlo:hhi, 128 * kb:128 * (kb + 1)],
                            qrope[pair][hlo:hhi, 512 * q:512 * (q + 1)],
                            start=True, stop=True)
                    ex = esb.tile([128, 2048], dt.bfloat16, tag="expp")
                    nc.scalar.activation(ex[:, :qlen], sc[:, :qlen], AF.Exp,
                                         scale=float(SCALE))
                    # causal mask on the diagonal 512-chunk:
                    # keep iff (512*qlo + x) - (128*kb + p) >= 0
                    nc.gpsimd.affine_select(
                        out=ex[:, 0:512], in_=ex[:, 0:512],
                        compare_op=ALU.is_ge, fill=0.0,
                        base=512 * qlo - 128 * kb,
                        pattern=[[1, 512]], channel_multiplier=-1)
                    for q in range(qlo, NS):
                        nc.tensor.matmul(
                            pvs[q][:],
                            vext[:, 128 * kb:128 * (kb + 1)],
                            ex[:, 512 * (q - qlo):512 * (q - qlo + 1)],
                            start=(kb == 0), stop=(kb == 4 * q + 3))
                # normalize: attnT[:, q] = pv / sumexp  (recip = exp(-ln(s)))
                for q in range(NS):
                    lns = nsb.tile([128, 512], dt.float32, tag="lns")
                    nc.scalar.activation(lns[smrow, :], pvs[q][smrow, :], AF.Ln)
                    lnd = nsb.tile([128, 512], dt.float32, tag="lnd")
                    nc.sync.dma_start(lnd[pvrow, :], lns[smrow, :])
                    rcp = nsb.tile([128, 512], dt.float32, tag="rcp")
                    nc.scalar.activation(rcp[pvrow, :], lnd[pvrow, :], AF.Exp,
                                         scale=-1.0)
                    nc.vector.tensor_tensor(
                        out=attnT[pair][hlo:hhi, 512 * q:512 * (q + 1)],
                        in0=pvs[q][pvrow, :], in1=rcp[pvrow, :], op=ALU.mult)

        # ---------------- o_proj partial:  poutT = woT.T @ attnT ----------------
        with tc.tile_pool(name="op_psum", bufs=2, space="PSUM") as opsum, \
             tc.tile_pool(name="out_sb", bufs=3) as osb:
            for m in range(KT):          # 16 tiles over the hidden (e) dim
                ps = opsum.tile([128, S], dt.float32, tag="ops")
                for kd in range(2):
                    for n in range(NS):
                        nc.tensor.matmul(
                            ps[:, 512 * n:512 * (n + 1)],
                            wo_b[kd][:, 128 * m:128 * (m + 1)],
                            attnT[kd][:, 512 * n:512 * (n + 1)],
                            start=(kd == 0), stop=(kd == 1))
                ob = osb.tile([128, S], dt.bfloat16, tag="ob")
                for n in range(NS):
                    sl = slice(512 * n, 512 * (n + 1))
                    if n % 2 == 0:
                        nc.vector.tensor_copy(ob[:, sl], ps[:, sl])
                    else:
                        nc.scalar.copy(ob[:, sl], ps[:, sl])
                nc.sync.dma_start(poutT[128 * m:128 * (m + 1), :], ob[:])

    _split_multi_waits(nc)
    return nc


_PROGRAM = None


def _get_program():
    global _PROGRAM
    if _PROGRAM is None:
        _PROGRAM = build_program()
    return _PROGRAM


# ---------------------------------------------------------------- host side
def make_inputs(hidden_states, position_ids, wq, wk, wv, wo):
    """Shard + marshal full inputs into per-core DRAM parameter maps."""
    x = np.asarray(hidden_states, dtype=np.float32).reshape(S, H)
    xT = np.ascontiguousarray(x.T)
    pos = np.asarray(position_ids).reshape(S).astype(np.float32)[None, :]
    inv_freq = (1.0 / (ROPE_BASE ** (np.arange(0, D, 2, dtype=np.float32) / D))
                ).astype(np.float32)[None, :]

    # rotation matrix RT2 [128, 128]: block-diag pair of RT [64, 64] where
    # (RT.T @ v)[j] = -v[j+32] for j<32, v[j-32] for j>=32  (rotate_half)
    R = np.zeros((D, D), dtype=np.float32)
    for j in range(32):
        R[j + 32, j] = -1.0       # out[j] = -in[j+32]
        R[j, j + 32] = 1.0        # out[j+32] = in[j]
    RT2 = np.zeros((128, 128), dtype=np.float32)
    RT2[0:64, 0:64] = R
    RT2[64:128, 64:128] = R

    wq = np.asarray(wq, dtype=np.float32)
    wk = np.asarray(wk, dtype=np.float32)
    wv = np.asarray(wv, dtype=np.float32)
    wo = np.asarray(wo, dtype=np.float32)

    in_maps = []
    for c in range(N_CORES):
        wq_c = wq[DQ * c:DQ * (c + 1)]           # [256, H]
        wk_c = wk[D * c:D * (c + 1)]             # [64, H]
        wv_c = wv[D * c:D * (c + 1)]             # [64, H]
        wqkvT_c = np.ascontiguousarray(
            np.concatenate([wq_c, wk_c, wv_c], axis=0).T)   # [H, 384]
        woT_c = np.ascontiguousarray(wo[:, DQ * c:DQ * (c + 1)].T)  # [256, H]
        in_maps.append({
            "xT": xT,
            "wqkvT": wqkvT_c,
            "woT": woT_c,
            "posr": pos,
            "invf": inv_freq,
            "rt2": RT2,
        })
    return in_maps


def kernel(hidden_states, position_ids, wq, wk, wv, wo):
    _install_profile_hook()
    nc = _get_program()
    in_maps = make_inputs(hidden_states, position_ids, wq, wk, wv, wo)
    res = run_bass_kernel_spmd(nc, in_maps, list(range(N_CORES)))
    acc = np.zeros((H, S), dtype=np.float32)
    for c in range(N_CORES):
        acc += res.results[c]["poutT"].astype(np.float32)
    return np.ascontiguousarray(acc.T)[None, :, :]


if __name__ == "__main__":
    rng = np.random.default_rng(0)
    hs = rng.standard_normal((1, S, H), dtype=np.float32)
    pid = np.broadcast_to(np.arange(S, dtype=np.int64)[None, :], (1, S))
    std = 1.0 / np.sqrt(H)
    w_q = (rng.standard_normal((NH * D, H), dtype=np.float32) * std)
    w_k = (rng.standard_normal((NKV * D, H), dtype=np.float32) * std)
    w_v = (rng.standard_normal((NKV * D, H), dtype=np.float32) * std)
    w_o = (rng.standard_normal((H, NH * D), dtype=np.float32) * std)
    out = kernel(hs, pid, w_q, w_k, w_v, w_o)
    print("out", out.shape, out.dtype, float(np.abs(out).mean()))


# revision 9
# speedup vs baseline: 1.2027x; 1.2027x over previous
"""Trainium2 Bass kernel for nn_Attention_28905129902499.

Dense transformer attention block (q/k/v proj + RoPE + causal GQA attention
+ o_proj), B=1, S=2048, HIDDEN=2048, 32 q heads / 8 kv heads, head_dim 64.

Sharding: tensor-parallel over heads across 8 NeuronCores. Core c owns
q heads 4c..4c+3 and kv head c. Each core computes its partial
out_c = attn_c @ wo[:, c*256:(c+1)*256].T  (shape [S, H]); the host sums the
8 partials (the tensor-parallel all-reduce) and returns the full output.

Device-side layout notes (per core):
  - All matmuls run in bf16 with fp32 PSUM accumulation.
  - q/k are produced *transposed*: qT/kT [d, s] with head_dim on partitions,
    so attention scores are computed directly transposed, scoresT[k, s] =
    kT.T @ qT, with no on-chip transposes of the big S x S tensors.
  - softmax runs without max subtraction (scores are O(+-6) here, exp is
    safe in fp32) and the denominators come for free out of the PV matmul:
    V is extended with 64 all-ones columns so out rows carry sum(exp).
  - RoPE cos/sin are computed on device from position_ids: freqs via a
    K=1 fp32 outer-product matmul, Cody-Waite range reduction on DVE,
    sin/cos on the ACT spline engine.
"""

import sys
import types
from contextlib import ExitStack

import numpy as np
import ml_dtypes

for _p in ("/opt/trn_rl_repo", "/root/.axon_site/_ro/trn_rl_repo"):
    if _p not in sys.path:
        sys.path.append(_p)

import concourse.bass as bass
import concourse.tile as tile
import concourse.mybir as mybir
from concourse.bass_utils import run_bass_kernel_spmd

dt = mybir.dt
AF = mybir.ActivationFunctionType
ALU = mybir.AluOpType
bf16 = ml_dtypes.bfloat16

# ---------------------------------------------------------------- constants
S = 2048          # sequence length
H = 2048          # hidden size
NH = 32           # query heads
NKV = 8           # kv heads
D = 64            # head dim
G = NH // NKV     # 4 query heads per kv head
N_CORES = 8
DQ = G * D        # 256 local q dims per core
MQKV = DQ + 2 * D   # 384 fused qkv output dims per core
KT = H // 128     # 16 contraction tiles
NS = S // 512     # 4 sequence chunks of 512
KB = S // 128     # 16 key blocks of 128
SCALE = 1.0 / np.sqrt(D)
ROPE_BASE = 10000.0

TWO_PI = 2.0 * np.pi
# Cody-Waite split of 2*pi for fp32 range reduction
_C1 = float(np.float32(np.ldexp(np.round(np.ldexp(TWO_PI, 11)), -11)))
_C2 = float(np.float32(np.ldexp(np.round(np.ldexp(TWO_PI - _C1, 23)), -23)))


def _split_multi_waits(nc):
    """The walrus build in this container accepts only ONE sync-wait per
    instruction; Tile emits more. Move extras onto same-engine NOPs placed
    immediately before the instruction (same-engine streams are in-order, so
    this is semantically identical)."""
    for bb in nc.main_func.blocks:
        insts = bb.instructions
        i = 0
        while i < len(insts):
            ins = insts[i]
            si = ins.sync_info
            waits = list(si.on_wait) if si is not None else []
            if len(waits) > 1:
                for w in waits[:-1]:
                    nop = mybir.InstNoOp(
                        name=nc.get_next_instruction_name(),
                        engine=ins.engine,
                        bass_nofuse=True,
                        sync_info=mybir.SyncInfo(on_wait=[w], on_update=[]),
                    )
                    nc.register_instruction(nop, overwrite=True)
                    insts.insert(i, nop)
                    i += 1
                ins.sync_info = mybir.SyncInfo(
                    on_wait=[waits[-1]], on_update=list(si.on_update)
                )
            i += 1


def _install_profile_hook():
    """Register the NTFF profile hook the agent image's antenv lacks, so
    run_bass_kernel_spmd(trace=True) can return HW exec times."""
    try:
        import antenv.axon_hooks  # noqa: F401
        return
    except ImportError:
        pass
    hook = None
    try:
        from trn_agent_boot.trn_boot import _ntff_profile_via_ctypes
        hook = _ntff_profile_via_ctypes("/opt/axon/libaxon_pjrt.so")
    except Exception:
        hook = None
    m = types.ModuleType("antenv.axon_hooks")
    m.get_axon_ntff_profile_hook = lambda: hook
    m.set_axon_ntff_profile_hook = lambda h: None
    sys.modules["antenv.axon_hooks"] = m


# ---------------------------------------------------------------- program
def build_program():
    nc = bass.Bass()

    xT = nc.declare_dram_parameter("xT", [H, S], dt.float32, isOutput=False)
    wqkvT = nc.declare_dram_parameter("wqkvT", [H, MQKV], dt.float32, isOutput=False)
    woT = nc.declare_dram_parameter("woT", [DQ, H], dt.float32, isOutput=False)
    posr = nc.declare_dram_parameter("posr", [1, S], dt.float32, isOutput=False)
    invf = nc.declare_dram_parameter("invf", [1, 32], dt.float32, isOutput=False)
    rt2 = nc.declare_dram_parameter("rt2", [128, 128], dt.float32, isOutput=False)
    poutT = nc.declare_dram_parameter("poutT", [H, S], dt.bfloat16, isOutput=True)

    with tile.TileContext(nc) as tc, ExitStack() as stack:
        # ---------------- persistent pools ----------------
        const_pool = stack.enter_context(tc.tile_pool(name="const", bufs=1))
        trig_pool = stack.enter_context(tc.tile_pool(name="trig", bufs=1))

        # pi/2 per-partition bias vector for cos-via-sin
        pi2_bias = const_pool.tile([128, 1], dt.float32, tag="pi2")
        nc.gpsimd.memset(pi2_bias[:], float(np.pi / 2))

        # rope rotation matrix
        rt_f = const_pool.tile([128, 128], dt.float32, tag="rtf")
        nc.sync.dma_start(rt_f[:], rt2[:])
        rt_b = const_pool.tile([128, 128], dt.bfloat16, tag="rtb")
        nc.gpsimd.tensor_copy(rt_b[:], rt_f[:])

        # position/frequency rows
        pos_sb = const_pool.tile([1, S], dt.float32, tag="pos")
        nc.sync.dma_start(pos_sb[:], posr[:])
        invf_sb = const_pool.tile([1, 32], dt.float32, tag="invf")
        nc.sync.dma_start(invf_sb[:], invf[:])

        # ---------------- RoPE trig tables ----------------
        # freqs in chunk-stacked layout [ (chunk c, f) , 512 ]:
        #   partition 32c+f  = inv_freq[f] * pos[512c + j]
        cos_rep = trig_pool.tile([128, S], dt.float32, tag="cosr")
        sin_rep = trig_pool.tile([128, S], dt.float32, tag="sinr")

        with tc.tile_pool(name="trig_psum", bufs=1, space="PSUM") as tpsum, \
             tc.tile_pool(name="trig_sc", bufs=1) as tsc:
            fq = tpsum.tile([128, 512], dt.float32, tag="fq")
            for c in range(4):
                nc.tensor.matmul(
                    fq[32 * c:32 * (c + 1), :],
                    invf_sb[:],
                    pos_sb[:, 512 * c:512 * (c + 1)],
                    start=True, stop=True,
                    tile_position=(0, 32 * c),
                )
            f_sb = tsc.tile([128, 512], dt.float32, tag="fsb")
            nc.vector.tensor_copy(f_sb[:], fq[:])

            # sin: k = round(f / 2pi); r = f - k*c1 - k*c2; sin(r)
            y = tsc.tile([128, 512], dt.float32, tag="y")
            nc.vector.tensor_scalar(out=y[:], in0=f_sb[:], scalar1=1.0 / TWO_PI,
                                    scalar2=None, op0=ALU.mult)
            ki = tsc.tile([128, 512], dt.int32, tag="ki")
            nc.vector.tensor_copy(ki[:], y[:])
            kf = tsc.tile([128, 512], dt.float32, tag="kf")
            nc.vector.tensor_copy(kf[:], ki[:])
            t1 = tsc.tile([128, 512], dt.float32, tag="t1")
            nc.vector.tensor_scalar(out=t1[:], in0=kf[:], scalar1=_C1,
                                    scalar2=None, op0=ALU.mult)
            r1 = tsc.tile([128, 512], dt.float32, tag="r1")
            nc.vector.tensor_tensor(out=r1[:], in0=f_sb[:], in1=t1[:], op=ALU.subtract)
            nc.vector.tensor_scalar(out=t1[:], in0=kf[:], scalar1=_C2,
                                    scalar2=None, op0=ALU.mult)
            nc.vector.tensor_tensor(out=r1[:], in0=r1[:], in1=t1[:], op=ALU.subtract)
            sin_c = tsc.tile([128, 512], dt.float32, tag="sinc")
            nc.scalar.activation(sin_c[:], r1[:], AF.Sin)

            # cos(f) = sin(f + pi/2 - kc*2pi), kc = round(f/2pi + 1/4)
            nc.vector.tensor_scalar(out=y[:], in0=y[:], scalar1=0.25,
                                    scalar2=None, op0=ALU.add)
            nc.vector.tensor_copy(ki[:], y[:])
            nc.vector.tensor_copy(kf[:], ki[:])
            nc.vector.tensor_scalar(out=t1[:], in0=kf[:], scalar1=_C1,
                                    scalar2=None, op0=ALU.mult)
            nc.vector.tensor_tensor(out=r1[:], in0=f_sb[:], in1=t1[:], op=ALU.subtract)
            nc.vector.tensor_scalar(out=t1[:], in0=kf[:], scalar1=_C2,
                                    scalar2=None, op0=ALU.mult)
            nc.vector.tensor_tensor(out=r1[:], in0=r1[:], in1=t1[:], op=ALU.subtract)
            cos_c = tsc.tile([128, 512], dt.float32, tag="cosc")
            nc.scalar.activation(cos_c[:], r1[:], AF.Sin, bias=pi2_bias[:])

            # replicate [ (c, f), 512 ] -> [ f rep x4 , (c, 512) ]
            for c in range(4):
                for i in range(4):
                    nc.sync.dma_start(
                        cos_rep[32 * i:32 * (i + 1), 512 * c:512 * (c + 1)],
                        cos_c[32 * c:32 * (c + 1), :])
                    nc.sync.dma_start(
                        sin_rep[32 * i:32 * (i + 1), 512 * c:512 * (c + 1)],
                        sin_c[32 * c:32 * (c + 1), :])

        # ---------------- load x / weights, convert to bf16 ----------------
        proj_pool = stack.enter_context(tc.tile_pool(name="proj", bufs=1))
        wqkv_b = [proj_pool.tile([128, MQKV], dt.bfloat16, tag=f"wqkv{k}", name=f"wqkv{k}")
                  for k in range(KT)]
        wo_b = [proj_pool.tile([128, S], dt.bfloat16, tag=f"wo{k}", name=f"wo{k}") for k in range(2)]
        xt_pool = stack.enter_context(tc.tile_pool(name="xtb", bufs=1))
        xt_b = [xt_pool.tile([128, S], dt.bfloat16, tag=f"xt{k}", name=f"xtb{k}") for k in range(KT)]

        with tc.tile_pool(name="stage", bufs=4) as stage:
            for k in range(KT):
                wf = stage.tile([128, MQKV], dt.float32, tag="wstage")
                nc.sync.dma_start(wf[:], wqkvT[128 * k:128 * (k + 1), :])
                if k % 2 == 0:
                    nc.vector.tensor_copy(wqkv_b[k][:], wf[:])
                else:
                    nc.scalar.copy(wqkv_b[k][:], wf[:])
                xf = stage.tile([128, S], dt.float32, tag="xstage")
                nc.sync.dma_start(xf[:], xT[128 * k:128 * (k + 1), :])
                if k % 2 == 0:
                    nc.scalar.copy(xt_b[k][:], xf[:])
                else:
                    nc.vector.tensor_copy(xt_b[k][:], xf[:])
            for k in range(2):
                wf2 = stage.tile([128, S], dt.float32, tag="wostage")
                nc.sync.dma_start(wf2[:], woT[128 * k:128 * (k + 1), :])
                nc.vector.tensor_copy(wo_b[k][:], wf2[:])

        # ---------------- fused QKV projection + RoPE ----------------
        # m=0: q heads 0,1 | m=1: q heads 2,3 | m=2: rows 0-63 kT, 64-127 vT
        att_pool = stack.enter_context(tc.tile_pool(name="att", bufs=1))
        qrope = [att_pool.tile([128, S], dt.bfloat16, tag=f"qrope{p}", name=f"qrope{p}")
                 for p in range(2)]
        krope = att_pool.tile([128, S], dt.bfloat16, tag="krope")
        # v_ext A: cols 0-63 v, 64-127 ones (pv rows 0-63, sums rows 64-127)
        # v_ext B: cols 0-63 ones, 64-127 v
        vextA = att_pool.tile([128, S], dt.bfloat16, tag="vextA")
        vextB = att_pool.tile([128, S], dt.bfloat16, tag="vextB")
        nc.gpsimd.memset(vextA[:], 1.0)
        nc.gpsimd.memset(vextB[:], 1.0)
        vT_sb = att_pool.tile([128, S], dt.bfloat16, tag="vTsb")

        with tc.tile_pool(name="qkv_psum", bufs=1, space="PSUM") as qpsum, \
             tc.tile_pool(name="rot_psum", bufs=2, space="PSUM") as rpsum, \
             tc.tile_pool(name="rope_sc", bufs=3) as rsc:
            for m in range(3):
                ps = qpsum.tile([128, S], dt.float32, tag="qkvps")
                for k in range(KT):
                    for n in range(NS):
                        nc.tensor.matmul(
                            ps[:, 512 * n:512 * (n + 1)],
                            wqkv_b[k][:, 128 * m:128 * (m + 1)],
                            xt_b[k][:, 512 * n:512 * (n + 1)],
                            start=(k == 0), stop=(k == KT - 1),
                        )
                nrows = 128 if m < 2 else 64
                for n in range(NS):
                    sl = slice(512 * n, 512 * (n + 1))
                    # qc = q * cos
                    qc = rsc.tile([128, 512], dt.float32, tag="qc")
                    nc.vector.tensor_tensor(out=qc[:nrows, :], in0=ps[:nrows, sl],
                                            in1=cos_rep[:nrows, sl], op=ALU.mult)
                    # raw copy for the rotation matmul
                    qraw = rsc.tile([128, 512], dt.bfloat16, tag="qraw")
                    nc.vector.tensor_copy(qraw[:nrows, :], ps[:nrows, sl])
                    rot = rpsum.tile([128, 512], dt.float32, tag="rot")
                    nc.tensor.matmul(rot[:nrows, :], rt_b[:nrows, :nrows],
                                     qraw[:nrows, :], start=True, stop=True)
                    # qs = rot(q) * sin ; qrope = qc + qs
                    qs = rsc.tile([128, 512], dt.float32, tag="qs")
                    nc.vector.tensor_tensor(out=qs[:nrows, :], in0=rot[:nrows, :],
                                            in1=sin_rep[:nrows, sl], op=ALU.mult)
                    dst = qrope[m] if m < 2 else krope
                    nc.vector.tensor_tensor(out=dst[:nrows, sl], in0=qc[:nrows, :],
                                            in1=qs[:nrows, :], op=ALU.add)
                    if m == 2:
                        # v rows: plain bf16 copy
                        nc.vector.tensor_copy(vT_sb[64:128, sl], ps[64:128, sl])

        # duplicate kT onto partitions 64-127 (for odd-head matmuls)
        nc.sync.dma_start(krope[64:128, :], krope[0:64, :])
        # transpose vT [64, S] into v_ext blocks [k(128), d(64)] per key block
        for kb in range(KB):
            nc.sync.dma_start_transpose(
                vextA[:, 128 * kb:128 * kb + 64],
                vT_sb[64:128, 128 * kb:128 * (kb + 1)])
        for kb in range(KB):
            nc.sync.dma_start(vextB[:, 128 * kb + 64:128 * (kb + 1)],
                              vextA[:, 128 * kb:128 * kb + 64])

        attnT = [att_pool.tile([128, S], dt.bfloat16, tag=f"attnT{p}", name=f"attnT{p}")
                 for p in range(2)]

        # ---------------- attention (per local q head) ----------------
        with tc.tile_pool(name="sc_psum", bufs=1, space="PSUM") as spsum, \
             tc.tile_pool(name="pv_psum", bufs=1, space="PSUM") as vpsum, \
             tc.tile_pool(name="exp_sb", bufs=2) as esb, \
             tc.tile_pool(name="norm_sb", bufs=4) as nsb:
            for h in range(4):
                pair = h // 2
                par = h % 2          # 0: even head (base 0), 1: odd (base 64)
                hlo, hhi = 64 * par, 64 * par + 64
                vext = vextA if par == 0 else vextB
                pvrow = slice(0, 64) if par == 0 else slice(64, 128)
                smrow = slice(64, 128) if par == 0 else slice(0, 64)
                pvs = [vpsum.tile([128, 512], dt.float32, tag=f"pv{q}", name=f"pv{q}")
                       for q in range(NS)]
                for kb in range(KB):
                    qlo = kb // 4        # first 512-chunk that attends to kb
                    # scores + exp in [128, <=1024] chunks (2-bank psum tiles,
                    # double-buffered so PE can run ahead of ACT)
                    exs = {}
                    for q0 in range(qlo, NS, 2):
                        qhi = min(q0 + 2, NS)
                        sc = spsum.tile([128, 1024], dt.float32, tag="scps",
                                        name="scps")
                        for q in range(q0, qhi):
                            nc.tensor.matmul(
                                sc[:, 512 * (q - q0):512 * (q - q0 + 1)],
                                krope[hlo:hhi, 128 * kb:128 * (kb + 1)],
                                qrope[pair][hlo:hhi, 512 * q:512 * (q + 1)],
                                start=True, stop=True)
                        ex = esb.tile([128, 1024], dt.bfloat16, tag="expp",
                                      name="expp")
                        qlen = 512 * (qhi - q0)
                        nc.scalar.activation(ex[:, :qlen], sc[:, :qlen], AF.Exp,
                                             scale=float(SCALE))
                        if q0 == qlo:
                            # causal mask on the diagonal 512-chunk:
                            # keep iff (512*qlo + x) - (128*kb + p) >= 0
                            nc.gpsimd.affine_select(
                                out=ex[:, 0:512], in_=ex[:, 0:512],
                                compare_op=ALU.is_ge, fill=0.0,
                                base=512 * qlo - 128 * kb,
                                pattern=[[1, 512]], channel_multiplier=-1)
                        for q in range(q0, qhi):
                            exs[q] = (ex, q - q0)
                    # PV: masked (diagonal) chunk last so its extra gpsimd
                    # dependency stays off the critical path
                    for q in list(range(qlo + 1, NS)) + [qlo]:
                        ex, off = exs[q]
                        nc.tensor.matmul(
                            pvs[q][:],
                            vext[:, 128 * kb:128 * (kb + 1)],
                            ex[:, 512 * off:512 * (off + 1)],
                            start=(kb == 0), stop=(kb == 4 * q + 3))
                # normalize: attnT[:, q] = pv / sumexp  (recip = exp(-ln(s)))
                for q in range(NS):
                    lns = nsb.tile([128, 512], dt.float32, tag="lns")
                    nc.scalar.activation(lns[smrow, :], pvs[q][smrow, :], AF.Ln)
                    lnd = nsb.tile([128, 512], dt.float32, tag="lnd")
                    nc.sync.dma_start(lnd[pvrow, :], lns[smrow, :])
                    rcp = nsb.tile([128, 512], dt.float32, tag="rcp")
                    nc.scalar.activation(rcp[pvrow, :], lnd[pvrow, :], AF.Exp,
                                         scale=-1.0)
                    nc.vector.tensor_tensor(
                        out=attnT[pair][hlo:hhi, 512 * q:512 * (q + 1)],
                        in0=pvs[q][pvrow, :], in1=rcp[pvrow, :], op=ALU.mult)

        # ---------------- o_proj partial:  poutT = woT.T @ attnT ----------------
        with tc.tile_pool(name="op_psum", bufs=2, space="PSUM") as opsum, \
             tc.tile_pool(name="out_sb", bufs=3) as osb:
            for m in range(KT):          # 16 tiles over the hidden (e) dim
                ps = opsum.tile([128, S], dt.float32, tag="ops")
                for kd in range(2):
                    for n in range(NS):
                        nc.tensor.matmul(
                            ps[:, 512 * n:512 * (n + 1)],
                            wo_b[kd][:, 128 * m:128 * (m + 1)],
                            attnT[kd][:, 512 * n:512 * (n + 1)],
                            start=(kd == 0), stop=(kd == 1))
                ob = osb.tile([128, S], dt.bfloat16, tag="ob")
                for n in range(NS):
                    sl = slice(512 * n, 512 * (n + 1))
                    if n % 2 == 0:
                        nc.vector.tensor_copy(ob[:, sl], ps[:, sl])
                    else:
                        nc.scalar.copy(ob[:, sl], ps[:, sl])
                nc.sync.dma_start(poutT[128 * m:128 * (m + 1), :], ob[:])

    _split_multi_waits(nc)
    return nc


_PROGRAM = None


def _get_program():
    global _PROGRAM
    if _PROGRAM is None:
        _PROGRAM = build_program()
    return _PROGRAM


# ---------------------------------------------------------------- host side
def make_inputs(hidden_states, position_ids, wq, wk, wv, wo):
    """Shard + marshal full inputs into per-core DRAM parameter maps."""
    x = np.asarray(hidden_states, dtype=np.float32).reshape(S, H)
    xT = np.ascontiguousarray(x.T)
    pos = np.asarray(position_ids).reshape(S).astype(np.float32)[None, :]
    inv_freq = (1.0 / (ROPE_BASE ** (np.arange(0, D, 2, dtype=np.float32) / D))
                ).astype(np.float32)[None, :]

    # rotation matrix RT2 [128, 128]: block-diag pair of RT [64, 64] where
    # (RT.T @ v)[j] = -v[j+32] for j<32, v[j-32] for j>=32  (rotate_half)
    R = np.zeros((D, D), dtype=np.float32)
    for j in range(32):
        R[j + 32, j] = -1.0       # out[j] = -in[j+32]
        R[j, j + 32] = 1.0        # out[j+32] = in[j]
    RT2 = np.zeros((128, 128), dtype=np.float32)
    RT2[0:64, 0:64] = R
    RT2[64:128, 64:128] = R

    wq = np.asarray(wq, dtype=np.float32)
    wk = np.asarray(wk, dtype=np.float32)
    wv = np.asarray(wv, dtype=np.float32)
    wo = np.asarray(wo, dtype=np.float32)

    in_maps = []
    for c in range(N_CORES):
        wq_c = wq[DQ * c:DQ * (c + 1)]           # [256, H]
        wk_c = wk[D * c:D * (c + 1)]             # [64, H]
        wv_c = wv[D * c:D * (c + 1)]             # [64, H]
        wqkvT_c = np.ascontiguousarray(
            np.concatenate([wq_c, wk_c, wv_c], axis=0).T)   # [H, 384]
        woT_c = np.ascontiguousarray(wo[:, DQ * c:DQ * (c + 1)].T)  # [256, H]
        in_maps.append({
            "xT": xT,
            "wqkvT": wqkvT_c,
            "woT": woT_c,
            "posr": pos,
            "invf": inv_freq,
            "rt2": RT2,
        })
    return in_maps


def kernel(hidden_states, position_ids, wq, wk, wv, wo):
    _install_profile_hook()
    nc = _get_program()
    in_maps = make_inputs(hidden_states, position_ids, wq, wk, wv, wo)
    res = run_bass_kernel_spmd(nc, in_maps, list(range(N_CORES)))
    acc = np.zeros((H, S), dtype=np.float32)
    for c in range(N_CORES):
        acc += res.results[c]["poutT"].astype(np.float32)
    return np.ascontiguousarray(acc.T)[None, :, :]


if __name__ == "__main__":
    rng = np.random.default_rng(0)
    hs = rng.standard_normal((1, S, H), dtype=np.float32)
    pid = np.broadcast_to(np.arange(S, dtype=np.int64)[None, :], (1, S))
    std = 1.0 / np.sqrt(H)
    w_q = (rng.standard_normal((NH * D, H), dtype=np.float32) * std)
    w_k = (rng.standard_normal((NKV * D, H), dtype=np.float32) * std)
    w_v = (rng.standard_normal((NKV * D, H), dtype=np.float32) * std)
    w_o = (rng.standard_normal((H, NH * D), dtype=np.float32) * std)
    out = kernel(hs, pid, w_q, w_k, w_v, w_o)
    print("out", out.shape, out.dtype, float(np.abs(out).mean()))


# revision 10
# speedup vs baseline: 1.3719x; 1.1407x over previous
"""Trainium2 Bass kernel for nn_Attention_28905129902499.

Dense transformer attention block (q/k/v proj + RoPE + causal GQA attention
+ o_proj), B=1, S=2048, HIDDEN=2048, 32 q heads / 8 kv heads, head_dim 64.

Sharding: tensor-parallel over heads across 8 NeuronCores. Core c owns
q heads 4c..4c+3 and kv head c. Each core computes its partial
out_c = attn_c @ wo[:, c*256:(c+1)*256].T  (shape [S, H]); the host sums the
8 partials (the tensor-parallel all-reduce) and returns the full output.

Device-side layout notes (per core):
  - All matmuls run in bf16 with fp32 PSUM accumulation.
  - q/k are produced *transposed*: qT/kT [d, s] with head_dim on partitions,
    so attention scores are computed directly transposed, scoresT[k, s] =
    kT.T @ qT, with no on-chip transposes of the big S x S tensors.
  - softmax runs without max subtraction (scores are O(+-6) here, exp is
    safe in fp32) and the denominators come for free out of the PV matmul:
    V is extended with 64 all-ones columns so out rows carry sum(exp).
  - RoPE cos/sin are computed on device from position_ids: freqs via a
    K=1 fp32 outer-product matmul, Cody-Waite range reduction on DVE,
    sin/cos on the ACT spline engine.
"""

import sys
import types
from contextlib import ExitStack

import numpy as np
import ml_dtypes

for _p in ("/opt/trn_rl_repo", "/root/.axon_site/_ro/trn_rl_repo"):
    if _p not in sys.path:
        sys.path.append(_p)

import concourse.bass as bass
import concourse.tile as tile
import concourse.mybir as mybir
from concourse.bass_utils import run_bass_kernel_spmd

dt = mybir.dt
AF = mybir.ActivationFunctionType
ALU = mybir.AluOpType
bf16 = ml_dtypes.bfloat16

# ---------------------------------------------------------------- constants
S = 2048          # sequence length
H = 2048          # hidden size
NH = 32           # query heads
NKV = 8           # kv heads
D = 64            # head dim
G = NH // NKV     # 4 query heads per kv head
N_CORES = 8
DQ = G * D        # 256 local q dims per core
MQKV = DQ + 2 * D   # 384 fused qkv output dims per core
KT = H // 128     # 16 contraction tiles
NS = S // 512     # 4 sequence chunks of 512
KB = S // 128     # 16 key blocks of 128
SCALE = 1.0 / np.sqrt(D)
ROPE_BASE = 10000.0

TWO_PI = 2.0 * np.pi
# Cody-Waite split of 2*pi for fp32 range reduction
_C1 = float(np.float32(np.ldexp(np.round(np.ldexp(TWO_PI, 11)), -11)))
_C2 = float(np.float32(np.ldexp(np.round(np.ldexp(TWO_PI - _C1, 23)), -23)))


def _split_multi_waits(nc):
    """The walrus build in this container accepts only ONE sync-wait per
    instruction; Tile emits more. Move extras onto same-engine NOPs placed
    immediately before the instruction (same-engine streams are in-order, so
    this is semantically identical)."""
    for bb in nc.main_func.blocks:
        insts = bb.instructions
        i = 0
        while i < len(insts):
            ins = insts[i]
            si = ins.sync_info
            waits = list(si.on_wait) if si is not None else []
            if len(waits) > 1:
                for w in waits[:-1]:
                    nop = mybir.InstNoOp(
                        name=nc.get_next_instruction_name(),
                        engine=ins.engine,
                        bass_nofuse=True,
                        sync_info=mybir.SyncInfo(on_wait=[w], on_update=[]),
                    )
                    nc.register_instruction(nop, overwrite=True)
                    insts.insert(i, nop)
                    i += 1
                ins.sync_info = mybir.SyncInfo(
                    on_wait=[waits[-1]], on_update=list(si.on_update)
                )
            i += 1


def _install_profile_hook():
    """Register the NTFF profile hook the agent image's antenv lacks, so
    run_bass_kernel_spmd(trace=True) can return HW exec times."""
    try:
        import antenv.axon_hooks  # noqa: F401
        return
    except ImportError:
        pass
    hook = None
    try:
        from trn_agent_boot.trn_boot import _ntff_profile_via_ctypes
        hook = _ntff_profile_via_ctypes("/opt/axon/libaxon_pjrt.so")
    except Exception:
        hook = None
    m = types.ModuleType("antenv.axon_hooks")
    m.get_axon_ntff_profile_hook = lambda: hook
    m.set_axon_ntff_profile_hook = lambda h: None
    sys.modules["antenv.axon_hooks"] = m


# ---------------------------------------------------------------- program
def build_program():
    nc = bass.Bass()

    xT = nc.declare_dram_parameter("xT", [H, S], dt.float32, isOutput=False)
    wqkvT = nc.declare_dram_parameter("wqkvT", [H, MQKV], dt.float32, isOutput=False)
    woT = nc.declare_dram_parameter("woT", [DQ, H], dt.float32, isOutput=False)
    posr = nc.declare_dram_parameter("posr", [1, S], dt.float32, isOutput=False)
    invf = nc.declare_dram_parameter("invf", [1, 32], dt.float32, isOutput=False)
    rt2 = nc.declare_dram_parameter("rt2", [128, 128], dt.float32, isOutput=False)
    poutT = nc.declare_dram_parameter("poutT", [H, S], dt.bfloat16, isOutput=True)

    with tile.TileContext(nc) as tc, ExitStack() as stack:
        # ---------------- persistent pools ----------------
        const_pool = stack.enter_context(tc.tile_pool(name="const", bufs=1))
        trig_pool = stack.enter_context(tc.tile_pool(name="trig", bufs=1))

        # pi/2 per-partition bias vector for cos-via-sin
        pi2_bias = const_pool.tile([128, 1], dt.float32, tag="pi2")
        nc.gpsimd.memset(pi2_bias[:], float(np.pi / 2))

        # rope rotation matrix
        rt_f = const_pool.tile([128, 128], dt.float32, tag="rtf")
        nc.sync.dma_start(rt_f[:], rt2[:])
        rt_b = const_pool.tile([128, 128], dt.bfloat16, tag="rtb")
        nc.gpsimd.tensor_copy(rt_b[:], rt_f[:])

        # position/frequency rows
        pos_sb = const_pool.tile([1, S], dt.float32, tag="pos")
        nc.sync.dma_start(pos_sb[:], posr[:])
        invf_sb = const_pool.tile([1, 32], dt.float32, tag="invf")
        nc.sync.dma_start(invf_sb[:], invf[:])

        # ---------------- load x / weights, convert to bf16 ----------------
        proj_pool = stack.enter_context(tc.tile_pool(name="proj", bufs=1))
        wqkv_b = [proj_pool.tile([128, MQKV], dt.bfloat16, tag=f"wqkv{k}", name=f"wqkv{k}")
                  for k in range(KT)]
        wo_b = [proj_pool.tile([128, S], dt.bfloat16, tag=f"wo{k}", name=f"wo{k}") for k in range(2)]
        xt_pool = stack.enter_context(tc.tile_pool(name="xtb", bufs=1))
        xt_b = [xt_pool.tile([128, S], dt.bfloat16, tag=f"xt{k}", name=f"xtb{k}") for k in range(KT)]
        with tc.tile_pool(name="stage", bufs=4) as stage:
            for k in range(KT):
                wf = stage.tile([128, MQKV], dt.float32, tag="wstage")
                nc.sync.dma_start(wf[:], wqkvT[128 * k:128 * (k + 1), :])
                nc.gpsimd.tensor_copy(wqkv_b[k][:], wf[:])
                xf = stage.tile([128, S], dt.float32, tag="xstage")
                # alternate HWDGE queues (sync / scalar) so 1MB loads overlap
                if k % 2 == 0:
                    nc.sync.dma_start(xf[:], xT[128 * k:128 * (k + 1), :])
                else:
                    nc.scalar.dma_start(xf[:], xT[128 * k:128 * (k + 1), :])
                nc.vector.tensor_copy(xt_b[k][:], xf[:])
            for k in range(2):
                wf2 = stage.tile([128, S], dt.float32, tag="wostage")
                nc.scalar.dma_start(wf2[:], woT[128 * k:128 * (k + 1), :])
                nc.vector.tensor_copy(wo_b[k][:], wf2[:])

        # ---------------- RoPE trig tables ----------------
        # freqs in chunk-stacked layout [ (chunk c, f) , 512 ]:
        #   partition 32c+f  = inv_freq[f] * pos[512c + j]
        cos_rep = trig_pool.tile([128, S], dt.float32, tag="cosr")
        sin_rep = trig_pool.tile([128, S], dt.float32, tag="sinr")

        with tc.tile_pool(name="trig_psum", bufs=1, space="PSUM") as tpsum, \
             tc.tile_pool(name="trig_sc", bufs=1) as tsc:
            fq = tpsum.tile([128, 512], dt.float32, tag="fq")
            for c in range(4):
                nc.tensor.matmul(
                    fq[32 * c:32 * (c + 1), :],
                    invf_sb[:],
                    pos_sb[:, 512 * c:512 * (c + 1)],
                    start=True, stop=True,
                    tile_position=(0, 32 * c),
                )
            f_sb = tsc.tile([128, 512], dt.float32, tag="fsb")
            nc.vector.tensor_copy(f_sb[:], fq[:])

            # sin: k = round(f / 2pi); r = f - k*c1 - k*c2; sin(r)
            y = tsc.tile([128, 512], dt.float32, tag="y")
            nc.vector.tensor_scalar(out=y[:], in0=f_sb[:], scalar1=1.0 / TWO_PI,
                                    scalar2=None, op0=ALU.mult)
            ki = tsc.tile([128, 512], dt.int32, tag="ki")
            nc.vector.tensor_copy(ki[:], y[:])
            kf = tsc.tile([128, 512], dt.float32, tag="kf")
            nc.vector.tensor_copy(kf[:], ki[:])
            t1 = tsc.tile([128, 512], dt.float32, tag="t1")
            nc.vector.tensor_scalar(out=t1[:], in0=kf[:], scalar1=_C1,
                                    scalar2=None, op0=ALU.mult)
            r1 = tsc.tile([128, 512], dt.float32, tag="r1")
            nc.vector.tensor_tensor(out=r1[:], in0=f_sb[:], in1=t1[:], op=ALU.subtract)
            nc.vector.tensor_scalar(out=t1[:], in0=kf[:], scalar1=_C2,
                                    scalar2=None, op0=ALU.mult)
            nc.vector.tensor_tensor(out=r1[:], in0=r1[:], in1=t1[:], op=ALU.subtract)
            sin_c = tsc.tile([128, 512], dt.float32, tag="sinc")
            nc.scalar.activation(sin_c[:], r1[:], AF.Sin)

            # cos(f) = sin(f + pi/2 - kc*2pi), kc = round(f/2pi + 1/4)
            nc.vector.tensor_scalar(out=y[:], in0=y[:], scalar1=0.25,
                                    scalar2=None, op0=ALU.add)
            nc.vector.tensor_copy(ki[:], y[:])
            nc.vector.tensor_copy(kf[:], ki[:])
            nc.vector.tensor_scalar(out=t1[:], in0=kf[:], scalar1=_C1,
                                    scalar2=None, op0=ALU.mult)
            nc.vector.tensor_tensor(out=r1[:], in0=f_sb[:], in1=t1[:], op=ALU.subtract)
            nc.vector.tensor_scalar(out=t1[:], in0=kf[:], scalar1=_C2,
                                    scalar2=None, op0=ALU.mult)
            nc.vector.tensor_tensor(out=r1[:], in0=r1[:], in1=t1[:], op=ALU.subtract)
            cos_c = tsc.tile([128, 512], dt.float32, tag="cosc")
            nc.scalar.activation(cos_c[:], r1[:], AF.Sin, bias=pi2_bias[:])

            # replicate [ (c, f), 512 ] -> [ f rep x4 , (c, 512) ]
            for c in range(4):
                for i in range(4):
                    nc.gpsimd.dma_start(
                        cos_rep[32 * i:32 * (i + 1), 512 * c:512 * (c + 1)],
                        cos_c[32 * c:32 * (c + 1), :])
                    nc.gpsimd.dma_start(
                        sin_rep[32 * i:32 * (i + 1), 512 * c:512 * (c + 1)],
                        sin_c[32 * c:32 * (c + 1), :])




        # ---------------- fused QKV projection + RoPE ----------------
        # m=0: q heads 0,1 | m=1: q heads 2,3 | m=2: rows 0-63 kT, 64-127 vT
        att_pool = stack.enter_context(tc.tile_pool(name="att", bufs=1))
        qrope = [att_pool.tile([128, S], dt.bfloat16, tag=f"qrope{p}", name=f"qrope{p}")
                 for p in range(2)]
        krope = att_pool.tile([128, S], dt.bfloat16, tag="krope")
        # v_ext A: cols 0-63 v, 64-127 ones (pv rows 0-63, sums rows 64-127)
        # v_ext B: cols 0-63 ones, 64-127 v
        vextA = att_pool.tile([128, S], dt.bfloat16, tag="vextA")
        vextB = att_pool.tile([128, S], dt.bfloat16, tag="vextB")
        nc.gpsimd.memset(vextA[:], 1.0)
        nc.gpsimd.memset(vextB[:], 1.0)
        vT_sb = att_pool.tile([128, S], dt.bfloat16, tag="vTsb")

        with tc.tile_pool(name="qkv_psum", bufs=1, space="PSUM") as qpsum, \
             tc.tile_pool(name="rot_psum", bufs=2, space="PSUM") as rpsum, \
             tc.tile_pool(name="rope_sc", bufs=3) as rsc:
            for m in range(3):
                ps = qpsum.tile([128, S], dt.float32, tag="qkvps")
                for k in range(KT):
                    for n in range(NS):
                        nc.tensor.matmul(
                            ps[:, 512 * n:512 * (n + 1)],
                            wqkv_b[k][:, 128 * m:128 * (m + 1)],
                            xt_b[k][:, 512 * n:512 * (n + 1)],
                            start=(k == 0), stop=(k == KT - 1),
                        )
                nrows = 128 if m < 2 else 64
                for n in range(NS):
                    sl = slice(512 * n, 512 * (n + 1))
                    # qc = q * cos
                    qc = rsc.tile([128, 512], dt.float32, tag="qc")
                    nc.vector.tensor_tensor(out=qc[:nrows, :], in0=ps[:nrows, sl],
                                            in1=cos_rep[:nrows, sl], op=ALU.mult)
                    # raw copy for the rotation matmul
                    qraw = rsc.tile([128, 512], dt.bfloat16, tag="qraw")
                    nc.vector.tensor_copy(qraw[:nrows, :], ps[:nrows, sl])
                    rot = rpsum.tile([128, 512], dt.float32, tag="rot")
                    nc.tensor.matmul(rot[:nrows, :], rt_b[:nrows, :nrows],
                                     qraw[:nrows, :], start=True, stop=True)
                    # qs = rot(q) * sin ; qrope = qc + qs
                    qs = rsc.tile([128, 512], dt.float32, tag="qs")
                    nc.vector.tensor_tensor(out=qs[:nrows, :], in0=rot[:nrows, :],
                                            in1=sin_rep[:nrows, sl], op=ALU.mult)
                    dst = qrope[m] if m < 2 else krope
                    nc.vector.tensor_tensor(out=dst[:nrows, sl], in0=qc[:nrows, :],
                                            in1=qs[:nrows, :], op=ALU.add)
                    if m == 2:
                        # v rows: plain bf16 copy
                        nc.vector.tensor_copy(vT_sb[64:128, sl], ps[64:128, sl])

        # duplicate kT onto partitions 64-127 (for odd-head matmuls)
        nc.gpsimd.dma_start(krope[64:128, :], krope[0:64, :])
        # transpose vT [64, S] into v_ext blocks [k(128), d(64)] per key block
        for kb in range(KB):
            nc.sync.dma_start_transpose(
                vextA[:, 128 * kb:128 * kb + 64],
                vT_sb[64:128, 128 * kb:128 * (kb + 1)])
        for kb in range(KB):
            nc.gpsimd.dma_start(vextB[:, 128 * kb + 64:128 * (kb + 1)],
                              vextA[:, 128 * kb:128 * kb + 64])

        attnT = [att_pool.tile([128, S], dt.bfloat16, tag=f"attnT{p}", name=f"attnT{p}")
                 for p in range(2)]

        # ---------------- attention (per local q head) ----------------
        with tc.tile_pool(name="sc_psum", bufs=2, space="PSUM") as spsum, \
             tc.tile_pool(name="pv_psum", bufs=1, space="PSUM") as vpsum, \
             tc.tile_pool(name="exp_sb", bufs=3) as esb, \
             tc.tile_pool(name="norm_sb", bufs=4) as nsb:
            for h in range(4):
                pair = h // 2
                par = h % 2          # 0: even head (base 0), 1: odd (base 64)
                hlo, hhi = 64 * par, 64 * par + 64
                vext = vextA if par == 0 else vextB
                pvrow = slice(0, 64) if par == 0 else slice(64, 128)
                smrow = slice(64, 128) if par == 0 else slice(0, 64)
                pvs = [vpsum.tile([128, 512], dt.float32, tag=f"pv{q}", name=f"pv{q}")
                       for q in range(NS)]
                for kb in range(KB):
                    qlo = kb // 4        # first 512-chunk that attends to kb
                    # scores + exp in [128, <=1024] chunks (2-bank psum tiles,
                    # double-buffered so PE can run ahead of ACT)
                    exs = {}
                    for q0 in range(qlo, NS, 2):
                        qhi = min(q0 + 2, NS)
                        sc = spsum.tile([128, 1024], dt.float32, tag="scps",
                                        name="scps")
                        for q in range(q0, qhi):
                            nc.tensor.matmul(
                                sc[:, 512 * (q - q0):512 * (q - q0 + 1)],
                                krope[hlo:hhi, 128 * kb:128 * (kb + 1)],
                                qrope[pair][hlo:hhi, 512 * q:512 * (q + 1)],
                                start=True, stop=True)
                        ex = esb.tile([128, 1024], dt.bfloat16, tag="expp",
                                      name="expp")
                        qlen = 512 * (qhi - q0)
                        nc.scalar.activation(ex[:, :qlen], sc[:, :qlen], AF.Exp,
                                             scale=float(SCALE))
                        if q0 == qlo:
                            # causal mask on the diagonal 512-chunk:
                            # keep iff (512*qlo + x) - (128*kb + p) >= 0
                            nc.gpsimd.affine_select(
                                out=ex[:, 0:512], in_=ex[:, 0:512],
                                compare_op=ALU.is_ge, fill=0.0,
                                base=512 * qlo - 128 * kb,
                                pattern=[[1, 512]], channel_multiplier=-1)
                        for q in range(q0, qhi):
                            exs[q] = (ex, q - q0)
                    # PV: masked (diagonal) chunk last so its extra gpsimd
                    # dependency stays off the critical path
                    for q in list(range(qlo + 1, NS)) + [qlo]:
                        ex, off = exs[q]
                        nc.tensor.matmul(
                            pvs[q][:],
                            vext[:, 128 * kb:128 * (kb + 1)],
                            ex[:, 512 * off:512 * (off + 1)],
                            start=(kb == 0), stop=(kb == 4 * q + 3))
                # normalize: attnT[:, q] = pv / sumexp  (recip = exp(-ln(s)))
                for q in range(NS):
                    lns = nsb.tile([128, 512], dt.float32, tag="lns")
                    nc.scalar.activation(lns[smrow, :], pvs[q][smrow, :], AF.Ln)
                    lnd = nsb.tile([128, 512], dt.float32, tag="lnd")
                    nc.sync.dma_start(lnd[pvrow, :], lns[smrow, :])
                    rcp = nsb.tile([128, 512], dt.float32, tag="rcp")
                    nc.scalar.activation(rcp[pvrow, :], lnd[pvrow, :], AF.Exp,
                                         scale=-1.0)
                    nc.vector.tensor_tensor(
                        out=attnT[pair][hlo:hhi, 512 * q:512 * (q + 1)],
                        in0=pvs[q][pvrow, :], in1=rcp[pvrow, :], op=ALU.mult)

        # ---------------- o_proj partial:  poutT = woT.T @ attnT ----------------
        with tc.tile_pool(name="op_psum", bufs=2, space="PSUM") as opsum, \
             tc.tile_pool(name="out_sb", bufs=3) as osb:
            for m in range(KT):          # 16 tiles over the hidden (e) dim
                ps = opsum.tile([128, S], dt.float32, tag="ops")
                for kd in range(2):
                    for n in range(NS):
                        nc.tensor.matmul(
                            ps[:, 512 * n:512 * (n + 1)],
                            wo_b[kd][:, 128 * m:128 * (m + 1)],
                            attnT[kd][:, 512 * n:512 * (n + 1)],
                            start=(kd == 0), stop=(kd == 1))
                ob = osb.tile([128, S], dt.bfloat16, tag="ob")
                for n in range(NS):
                    sl = slice(512 * n, 512 * (n + 1))
                    if n % 2 == 0:
                        nc.vector.tensor_copy(ob[:, sl], ps[:, sl])
                    else:
                        nc.scalar.copy(ob[:, sl], ps[:, sl])
                if m % 2 == 0:
                    nc.sync.dma_start(poutT[128 * m:128 * (m + 1), :], ob[:])
                else:
                    nc.scalar.dma_start(poutT[128 * m:128 * (m + 1), :], ob[:])

    _split_multi_waits(nc)
    return nc


_PROGRAM = None


def _get_program():
    global _PROGRAM
    if _PROGRAM is None:
        _PROGRAM = build_program()
    return _PROGRAM


# ---------------------------------------------------------------- host side
def make_inputs(hidden_states, position_ids, wq, wk, wv, wo):
    """Shard + marshal full inputs into per-core DRAM parameter maps."""
    x = np.asarray(hidden_states, dtype=np.float32).reshape(S, H)
    xT = np.ascontiguousarray(x.T)
    pos = np.asarray(position_ids).reshape(S).astype(np.float32)[None, :]
    inv_freq = (1.0 / (ROPE_BASE ** (np.arange(0, D, 2, dtype=np.float32) / D))
                ).astype(np.float32)[None, :]

    # rotation matrix RT2 [128, 128]: block-diag pair of RT [64, 64] where
    # (RT.T @ v)[j] = -v[j+32] for j<32, v[j-32] for j>=32  (rotate_half)
    R = np.zeros((D, D), dtype=np.float32)
    for j in range(32):
        R[j + 32, j] = -1.0       # out[j] = -in[j+32]
        R[j, j + 32] = 1.0        # out[j+32] = in[j]
    RT2 = np.zeros((128, 128), dtype=np.float32)
    RT2[0:64, 0:64] = R
    RT2[64:128, 64:128] = R

    wq = np.asarray(wq, dtype=np.float32)
    wk = np.asarray(wk, dtype=np.float32)
    wv = np.asarray(wv, dtype=np.float32)
    wo = np.asarray(wo, dtype=np.float32)

    in_maps = []
    for c in range(N_CORES):
        wq_c = wq[DQ * c:DQ * (c + 1)]           # [256, H]
        wk_c = wk[D * c:D * (c + 1)]             # [64, H]
        wv_c = wv[D * c:D * (c + 1)]             # [64, H]
        wqkvT_c = np.ascontiguousarray(
            np.concatenate([wq_c, wk_c, wv_c], axis=0).T)   # [H, 384]
        woT_c = np.ascontiguousarray(wo[:, DQ * c:DQ * (c + 1)].T)  # [256, H]
        in_maps.append({
            "xT": xT,
            "wqkvT": wqkvT_c,
            "woT": woT_c,
            "posr": pos,
            "invf": inv_freq,
            "rt2": RT2,
        })
    return in_maps


def kernel(hidden_states, position_ids, wq, wk, wv, wo):
    _install_profile_hook()
    nc = _get_program()
    in_maps = make_inputs(hidden_states, position_ids, wq, wk, wv, wo)
    res = run_bass_kernel_spmd(nc, in_maps, list(range(N_CORES)))
    acc = np.zeros((H, S), dtype=np.float32)
    for c in range(N_CORES):
        acc += res.results[c]["poutT"].astype(np.float32)
    return np.ascontiguousarray(acc.T)[None, :, :]


if __name__ == "__main__":
    rng = np.random.default_rng(0)
    hs = rng.standard_normal((1, S, H), dtype=np.float32)
    pid = np.broadcast_to(np.arange(S, dtype=np.int64)[None, :], (1, S))
    std = 1.0 / np.sqrt(H)
    w_q = (rng.standard_normal((NH * D, H), dtype=np.float32) * std)
    w_k = (rng.standard_normal((NKV * D, H), dtype=np.float32) * std)
    w_v = (rng.standard_normal((NKV * D, H), dtype=np.float32) * std)
    w_o = (rng.standard_normal((H, NH * D), dtype=np.float32) * std)
    out = kernel(hs, pid, w_q, w_k, w_v, w_o)
    print("out", out.shape, out.dtype, float(np.abs(out).mean()))
